# revision 5
# baseline (speedup 1.0000x reference)
"""Trainium2 Bass kernel for nn_MinSimilarityScorer.

Per batch episode b (independent; data-parallel over B=32 across 8 cores):
  tm    = mean_k test_reps[b,k]                  (T,D)
  sup   = support_reps[b] reshaped (S,D), S=K*L
  tgt   = support_targets[b] reshaped (S,NT)
  nn    = argmin_s ||tm[t]-sup[s]||^2  == argmax_s (tsum.sup - 5*||sup||^2)
          where tsum = K*tm (DMA-accumulated sum; scale folds out of argmax)
  out0  = tgt[nn] + 0.5 * (tm @ proto^T)
  proto = (tgt^T @ sup) / (count + 1e-4)         (NT,D)
"""

import sys

import numpy as np

for _p in ("/opt/trn_rl_repo",):
    if _p not in sys.path:
        sys.path.insert(0, _p)

import concourse.bass as bass
import concourse.mybir as mybir
import concourse.tile as tile
from concourse import bacc
from concourse.bass_utils import run_bass_kernel_spmd
from concourse.masks import make_identity

B, K, T, L, D, NT = 32, 10, 128, 128, 1024, 32
NCORES = 8
BPC = B // NCORES  # batches per core
S = K * L  # 1280 support rows
SQRT5 = float(np.sqrt(5.0))
F32 = mybir.dt.float32

# s-blocks of the 1280-wide score matrix (PSUM matmul free-dim <= 512)
SBLOCKS = [(0, 512), (512, 512), (1024, 256)]
# k-groups for assembling supT columns through a [128,512] staging psum tile
KGROUPS = [(0, (0, 1, 2, 3)), (1, (4, 5, 6, 7)), (2, (8, 9))]


def build_model(bpc: int = BPC) -> bass.Bass:
    nc = bacc.Bacc(None, target_bir_lowering=False, debug=False)
    test_d = nc.dram_tensor("test_reps", [bpc, K, T, D], F32, kind="ExternalInput")
    sup_d = nc.dram_tensor("support_reps", [bpc, K, L, D], F32, kind="ExternalInput")
    tgt_d = nc.dram_tensor("support_targets", [bpc, K, L, NT], F32, kind="ExternalInput")
    out0_d = nc.dram_tensor("out_score", [bpc, T, NT], F32, kind="ExternalOutput")
    out1_d = nc.dram_tensor("out_proto", [bpc, NT, D], F32, kind="ExternalOutput")

    with tile.TileContext(nc) as tc:
        with (
            tc.tile_pool(name="const", bufs=1) as constp,
            tc.tile_pool(name="sup", bufs=12) as supp,
            tc.tile_pool(name="big", bufs=2) as bigp,
            tc.tile_pool(name="supT", bufs=3) as supTp,
            tc.tile_pool(name="small", bufs=2) as smallp,
            tc.tile_pool(name="ps_stage", bufs=2, space="PSUM") as ps_stage,
            tc.tile_pool(name="ps_dot", bufs=1, space="PSUM") as ps_dot,
            tc.tile_pool(name="ps_proto", bufs=1, space="PSUM") as ps_proto,
            tc.tile_pool(name="ps_small", bufs=1, space="PSUM") as ps_small,
        ):
            ident = constp.tile([128, 128], F32, tag="ident")
            make_identity(nc, ident[:])
            neg1 = constp.tile([1, 128], F32, tag="neg1")
            nc.gpsimd.memset(neg1[:], -1.0)
            ones_col = constp.tile([128, 1], F32, tag="ones")
            nc.gpsimd.memset(ones_col[:], 1.0)

            for b in range(bpc):
                # ---------- loads ----------
                sup_sb = []
                for k in range(K):
                    t_ = supp.tile([L, D], F32, tag="sup")
                    nc.sync.dma_start(t_[:], sup_d[b, k])
                    sup_sb.append(t_)
                tgt_sb = smallp.tile([L, K, NT], F32, tag="tgt")
                nc.sync.dma_start(tgt_sb[:], tgt_d[b].rearrange("k l n -> l k n"))

                # tsum = sum_k test_reps[b,k] via accumulating DMA (SWDGE CCE add)
                tsum = bigp.tile([T, D], F32, tag="tsum")
                nc.gpsimd.dma_start(tsum[:], test_d[b, 0])
                for k in range(1, K):
                    nc.gpsimd.dma_start(
                        tsum[:], test_d[b, k], accum_op=mybir.AluOpType.add
                    )

                # ---------- support norms: 5*||sup[s]||^2 as a [1,S] row ----------
                norms_nat = smallp.tile([L, K], F32, tag="norms")
                normrow = smallp.tile([1, S], F32, tag="normrow")
                for k in range(K):
                    sq = bigp.tile([L, D], F32, tag="sq")
                    nc.scalar.activation(
                        sq[:],
                        sup_sb[k][:],
                        mybir.ActivationFunctionType.Square,
                        scale=SQRT5,
                        accum_out=norms_nat[:, k : k + 1],
                    )
                    # flip [L,1] column (partition dim) into the [1,L] row chunk
                    nc.sync.dma_start(
                        normrow[:, 128 * k : 128 * (k + 1)], norms_nat[:, k : k + 1]
                    )

                # ---------- prototypes: tgt^T @ sup, counts, scale ----------
                counts_ps = ps_small.tile([NT, 1], F32, tag="cnt")
                proto_sb = smallp.tile([NT, D], F32, tag="proto")
                inv_cnt = smallp.tile([NT, 1], F32, tag="inv")
                for half in range(2):
                    pps = ps_proto.tile([NT, 512], F32, tag="pps")
                    for k in range(K):
                        nc.tensor.matmul(
                            pps[:],
                            lhsT=tgt_sb[:, k, :],
                            rhs=sup_sb[k][:, 512 * half : 512 * (half + 1)],
                            start=(k == 0),
                            stop=(k == K - 1),
                        )
                        if half == 0:
                            nc.tensor.matmul(
                                counts_ps[:],
                                lhsT=tgt_sb[:, k, :],
                                rhs=ones_col[:],
                                start=(k == 0),
                                stop=(k == K - 1),
                            )
                    if half == 0:
                        nc.vector.tensor_scalar(
                            out=inv_cnt[:],
                            in0=counts_ps[:],
                            scalar1=1e-4,
                            scalar2=None,
                            op0=mybir.AluOpType.add,
                        )
                        nc.vector.reciprocal(inv_cnt[:], inv_cnt[:])
                    nc.vector.tensor_scalar(
                        out=proto_sb[:, 512 * half : 512 * (half + 1)],
                        in0=pps[:],
                        scalar1=inv_cnt[:, 0:1],
                        scalar2=None,
                        op0=mybir.AluOpType.mult,
                    )
                nc.sync.dma_start(out1_d[b], proto_sb[:])

                # ---------- tmT: transpose tsum -> [d, t] chunks ----------
                tmT = bigp.tile([128, D], F32, tag="tmT")
                for jg in range(2):
                    st = ps_stage.tile([128, 512], F32, tag="st")
                    for m in range(4):
                        j = 4 * jg + m
                        nc.tensor.transpose(
                            st[:, 128 * m : 128 * (m + 1)],
                            tsum[:, 128 * j : 128 * (j + 1)],
                            ident[:],
                        )
                    if jg == 0:
                        nc.vector.tensor_copy(tmT[:, 0:512], st[:])
                    else:
                        nc.scalar.copy(tmT[:, 512:1024], st[:])

                # ---------- dots: tsum . sup  accumulated over d-chunks ----------
                dots = ps_dot.tile([T, S], F32, tag="dots")
                for j in range(8):
                    supT = supTp.tile([128, S], F32, tag="supT")
                    for kg, ks in KGROUPS:
                        st = ps_stage.tile([128, 512], F32, tag="st")
                        for i, k in enumerate(ks):
                            nc.tensor.transpose(
                                st[:, 128 * i : 128 * (i + 1)],
                                sup_sb[k][:, 128 * j : 128 * (j + 1)],
                                ident[:],
                            )
                        width = 128 * len(ks)
                        if (j + kg) % 2 == 0:
                            nc.vector.tensor_copy(
                                supT[:, 512 * kg : 512 * kg + width], st[:, :width]
                            )
                        else:
                            nc.scalar.copy(
                                supT[:, 512 * kg : 512 * kg + width], st[:, :width]
                            )
                    for s0, sw in SBLOCKS:
                        nc.tensor.matmul(
                            dots[:, s0 : s0 + sw],
                            lhsT=tmT[:, 128 * j : 128 * (j + 1)],
                            rhs=supT[:, s0 : s0 + sw],
                            start=(j == 0),
                            stop=False,
                        )
                # subtract 5*||sup||^2 broadcast over t via K=1 matmul with -1 row
                for s0, sw in SBLOCKS:
                    nc.tensor.matmul(
                        dots[:, s0 : s0 + sw],
                        lhsT=neg1[:],
                        rhs=normrow[:, s0 : s0 + sw],
                        start=False,
                        stop=True,
                    )

                # ---------- argmax + gather ----------
                scores = bigp.tile([T, S], F32, tag="scores")
                nc.scalar.copy(scores[:], dots[:])
                mx8 = smallp.tile([T, 8], F32, tag="mx8")
                idx8 = smallp.tile([T, 8], mybir.dt.uint32, tag="idx8")
                nc.vector.max(mx8[:], scores[:])
                nc.vector.max_index(idx8[:], mx8[:], scores[:])
                gath = smallp.tile([T, NT], F32, tag="gath")
                nc.gpsimd.indirect_dma_start(
                    out=gath[:],
                    out_offset=None,
                    in_=tgt_d[:].rearrange("b k l n -> (b k l) n"),
                    in_offset=bass.IndirectOffsetOnAxis(ap=idx8[:, 0:1], axis=0),
                    element_offset=b * S * NT,
                )

                # ---------- sim1 = tsum @ proto^T ----------
                protoT = smallp.tile([128, 256], F32, tag="protoT")
                for jg in range(2):
                    st = ps_stage.tile([128, 512], F32, tag="st")
                    for m in range(4):
                        j = 4 * jg + m
                        nc.tensor.transpose(
                            st[:, 32 * m : 32 * (m + 1)],
                            proto_sb[:, 128 * j : 128 * (j + 1)],
                            ident[:NT, :NT],
                        )
                    nc.vector.tensor_copy(
                        protoT[:, 128 * jg : 128 * (jg + 1)], st[:, :128]
                    )
                sim1 = ps_small.tile([T, NT], F32, tag="sim1")
                for j in range(8):
                    nc.tensor.matmul(
                        sim1[:],
                        lhsT=tmT[:, 128 * j : 128 * (j + 1)],
                        rhs=protoT[:, 32 * j : 32 * (j + 1)],
                        start=(j == 0),
                        stop=(j == 7),
                    )
                sim1_sb = smallp.tile([T, NT], F32, tag="sim1sb")
                nc.scalar.mul(sim1_sb[:], sim1[:], 0.05)
                out0_sb = smallp.tile([T, NT], F32, tag="out0")
                nc.vector.tensor_add(out0_sb[:], gath[:], sim1_sb[:])
                nc.sync.dma_start(out0_d[b], out0_sb[:])

    nc.compile()
    return nc


_MODEL = None
LAST_RESULTS = None


def _get_model():
    global _MODEL
    if _MODEL is None:
        _MODEL = build_model()
    return _MODEL


def _run(inputs: dict, trace: bool = False):
    global LAST_RESULTS
    nc = _get_model()
    test = np.ascontiguousarray(np.asarray(inputs["test_reps"], dtype=np.float32))
    sup = np.ascontiguousarray(np.asarray(inputs["support_reps"], dtype=np.float32))
    tgt = np.ascontiguousarray(np.asarray(inputs["support_targets"], dtype=np.float32))
    in_maps = []
    for c in range(NCORES):
        sl = slice(c * BPC, (c + 1) * BPC)
        in_maps.append(
            {
                "test_reps": np.ascontiguousarray(test[sl]),
                "support_reps": np.ascontiguousarray(sup[sl]),
                "support_targets": np.ascontiguousarray(tgt[sl]),
            }
        )
    res = run_bass_kernel_spmd(
        nc, in_maps, core_ids=list(range(NCORES)), trace=trace
    )
    LAST_RESULTS = res
    out0 = np.concatenate([res.results[c]["out_score"] for c in range(NCORES)], axis=0)
    out1 = np.concatenate([res.results[c]["out_proto"] for c in range(NCORES)], axis=0)
    return out0, out1


def kernel(
    test_reps,
    support_reps,
    test_output_mask=None,
    support_output_mask=None,
    support_targets=None,
):
    # masks are all-ones and unused by the reference computation
    out0, out1 = _run(
        {
            "test_reps": test_reps,
            "support_reps": support_reps,
            "support_targets": support_targets,
        }
    )
    return out0, out1
